# revision 19
# baseline (speedup 1.0000x reference)
"""Trainium2 Bass kernel for nn_Attention_53386443489626.

Math (per batch b):
    fkeys = W_fk @ field + b_fk          [NK, Lf]
    fvals = W_fv @ field + b_fv          [NV, Lf]
    hkeys = W_qk @ query + b_qk          [NK, Lq]
    z     = fkeys^T @ hkeys / sqrt(NK)   [Lf, Lq]
    w     = exp(clip(z, -30, 30))        (clip is a no-op: max |z| ~ 9.4)
    w     = w / sum_l w
    y     = fvals @ w                    [NV, Lq]

Single-pass accumulation (no running max needed; clip bounds the exponent):
    acc[v,q] = sum_l fvals0[v,l] * wu[l,q],  den[q] = sum_l wu[l,q]
    y[v,q]   = acc[v,q] / den[q] + b_fv[v]
acc and den come from ONE PE matmul stream by appending a ones column to the
transposed values (fvT[:, 65th] = 1).

Sharding: 8 cores = 4 batches x 2 query-halves; normalization is over Lf so
no cross-core communication.

Perf structure (v2): the baseline was ACT-bound -- exp of 8.4M score
elements/core at 1 el/lane/cycle is ~70us on the scalar engine alone. This
version splits the exp across TWO engines per l-tile pair:
  - ACT pairs: native nc.scalar.activation(Exp)   (~1.04us / [128,1024] tile)
  - DVE pairs: Schraudolph bit-trick exp in ONE tensor_scalar op:
        i32 = int32(z * (2^23*log2(e)/8) + 2^23*(127 - c))
    whose bit pattern read as f32 approximates exp(z/8) to ~3% max rel err
    (validated end-to-end: adds ~3e-3 to the final y rel err, tolerance 2e-2).
The K=64 score matmuls stay row-group packed (two concurrent row groups via
tile_position), f32r everywhere (bf16-rate on PE). Epilogue is split across
ACT/DVE and its emission deferred into the next q-block so the engine FIFOs
don't stall on the accumulation tail.
"""

import numpy as np
from contextlib import ExitStack

try:
    import concourse  # noqa: F401
except ImportError:  # pragma: no cover
    import sys

    sys.path.insert(0, "/opt/trn_rl_repo")

import concourse.bacc as bacc
import concourse.mybir as mybir
import concourse.tile as tile
from concourse.bass_utils import run_bass_kernel_spmd

dt = mybir.dt
AF = mybir.ActivationFunctionType
ALU = mybir.AluOpType

B, NF, NK, NV = 4, 128, 64, 64
LF, LQ = 4096, 4096
import os as _os

NCORES = 8
QSH = NCORES // B  # query shards per batch = 2
LQS = LQ // QSH  # per-core query length = 2048
NLT = LF // 128  # 32 l-tiles
NPAIR = NLT // 2  # 16 l-tile pairs
QB = 512  # query columns per accumulation block
NQB = LQS // QB  # 4
SCALE = 1.0 / np.sqrt(NK)  # 0.125

N_ACT = int(_os.environ.get("KACT", "9"))  # pairs on ACT (rest on DVE)
DEPTH = int(_os.environ.get("KDEPTH", "2"))  # acc-matmul pipeline depth
Q2 = _os.environ.get("KQ2", "1") == "1"  # use the ACT hwdge queue too
DUMP = _os.environ.get("KDUMP", "0") == "1"  # debug: dump intermediates
ABL = _os.environ.get("KABL", "")  # timing ablations: zaconly | noacc
PACK = _os.environ.get("KPACK", "1") == "1"  # row-group-packed z matmuls
FWL = _os.environ.get("KFWL", "0") == "1"  # pad acc stationary to 128 (FWL)
EPID = _os.environ.get("KEPID", "1") == "1"  # defer epilogue into next q-block
GPS = _os.environ.get("KGPS", "1") == "1"  # y2 bias-add on gpsimd
SCHRAUD_C = float(_os.environ.get("KC", "0.0434"))
# 16-bit Schraudolph: bf16 bits = 128*(z*log2(e)*scale + 127 - c), emitted as
# int16 and bitcast to bf16 (the f32 variant is rejected by the BIR verifier:
# f32r-matmul inputs must be "rounded to FP32r")
A_CONST = float(2.0**7 * np.log2(np.e) * SCALE)
B_CONST = float(2.0**7 * (127.0 - SCHRAUD_C) + 0.5)

ACT_SET = {(i * NPAIR) // N_ACT for i in range(N_ACT)} if N_ACT > 0 else set()


def emit_body(nc, tc, io, p):
    f32 = dt.float32
    f32r = dt.float32r

    # ---- constants ------------------------------------------------------
    # weights packed as one [128, 3*64] dram tensor, biases as one [64, 3]
    # (bias columns side by side on partitions 0-63: per-partition bias APs
    # must live on the SAME partitions as the outputs they bias)
    wpack = p["const"].tile([NF, 3 * NK], f32r, tag="wpack")
    bpack = p["const"].tile([NV + 1, 3], f32, tag="bpack")
    wfkT = wpack[:, 0:NK]
    wqkT = wpack[:, NK : 2 * NK]
    wfvT = wpack[:, 2 * NK : 3 * NK]
    bfk = bpack[0:NK, 0:1]
    bqk = bpack[0:NK, 1:2]
    bfv65 = bpack[:, 2:3]  # [65,1]: row 0 pad, rows 1..64 = b_fv
    ones65 = p["const"].tile([1, NV + 1], f32, tag="ones65")
    nc.vector.memset(ones65, 1.0)

    # ---- inputs (chunked so early projections unblock early) ------------
    # Each DMA occupies a serialized ~650ns HWDGE slot, so order matters and
    # the two hardware queues (SP + Activation) are both used: the first
    # field/query chunks go FIRST so the projections can start ~2us in.
    fieldT = [
        p["big"].tile([NF, 1024], f32r, tag=f"field{c}", name=f"field{c}")
        for c in range(LF // 1024)
    ]
    queryT = [
        p["big"].tile([NF, 1024], f32r, tag=f"query{c}", name=f"query{c}")
        for c in range(LQS // 1024)
    ]
    def half_loads(dq, tiles, src, idx):
        for c, h in idx:
            dq.dma_start(
                out=tiles[c][:, h * 512 : (h + 1) * 512],
                in_=src[:, c * 1024 + h * 512 : c * 1024 + (h + 1) * 512],
            )

    dq2 = nc.scalar if Q2 else nc.sync
    half_loads(nc.sync, fieldT, io["field"], [(0, 0)])
    half_loads(dq2, queryT, io["query"], [(0, 0)])
    nc.sync.dma_start(out=wpack, in_=io["wpack"])
    dq2.dma_start(out=bpack, in_=io["bpack"])
    half_loads(nc.sync, fieldT, io["field"], [(0, 1)])
    half_loads(dq2, queryT, io["query"], [(0, 1)])
    half_loads(nc.sync, fieldT, io["field"], [(1, 0), (1, 1)])
    half_loads(dq2, queryT, io["query"], [(1, 0), (1, 1)])
    half_loads(nc.sync, fieldT, io["field"], [(2, 0), (2, 1), (3, 0), (3, 1)])

    # fkeys2[0:64, pr*128+i]   = fkeys[k, (2*pr)*128+i]   (even l-tiles, top)
    # fkeys2[64:128, pr*128+i] = fkeys[k, (2*pr+1)*128+i] (odd l-tiles, bottom)
    # Keys in bf16: halves the duplication-DMA bytes and SBUF traffic, and
    # bf16 weights get the fast (FWL) LDWEIGHTS path. Validated: adds <1e-3
    # to the final rel err on top of the approx-exp share.
    bf16 = dt.bfloat16
    fkeys2 = p["big"].tile([128, NPAIR * 128], bf16, tag="fkeys")
    hkeys2 = p["big"].tile([128, LQS], bf16, tag="hkeys")  # duplicated halves
    fkeys = p["big"].tile([NK, LF], bf16, tag="fkeysflat")
    # ones in COLUMN 0 (not 64): the denominator lands on acc partition 0,
    # where reciprocal_approx_fast can read it (the custom DVE op ignores a
    # nonzero partition-base offset on HW -- it read acc row 0 regardless)
    NVC = 128 if FWL else NV + 1  # pad stationary to 128 cols for FWL
    fvT = p["big"].tile([128, NLT, NVC], bf16, tag="fvT")
    if FWL:
        nc.vector.memset(fvT[:, :, :], 0.0)
    nc.vector.memset(fvT[:, :, 0:1], 1.0)  # denominator column

    # trigger the ACT exp table load at t~0 (it costs ~2.7us; hide it under
    # the input DMAs instead of the first score tile)
    dume = p["const"].tile([1, 1], f32, tag="dume")
    nc.scalar.activation(out=dume, in_=ones65[0:1, 0:1], func=AF.Exp)

    def ztile():
        return p["z"].tile([128, 1024], f32, tag="z", name="zt")

    # ---- projections ----------------------------------------------------
    # Two 512-wide matmuls per PSUM tile; one combined bias+copy per tile,
    # alternating ACT/DVE so neither engine serializes the prologue.
    copy_eng = [0]

    def bias_copy(out, in_, bias):
        if copy_eng[0] % 2 == 0:
            nc.scalar.activation(out=out, in_=in_, func=AF.Identity, bias=bias)
        else:
            nc.vector.tensor_scalar(
                out=out, in0=in_, scalar1=bias, scalar2=None, op0=ALU.add
            )
        copy_eng[0] += 1

    def emit_fk_chunk(jt, dq):  # 1024 cols of fkeys + its fkeys2 duplication
        zt = ztile()
        for h in range(2):
            j = 2 * jt + h
            nc.tensor.matmul(
                zt[:NK, h * 512 : (h + 1) * 512],
                wfkT,
                fieldT[j // 2][:, (j % 2) * 512 : (j % 2) * 512 + 512],
                start=True,
                stop=True,
            )
        bias_copy(fkeys[:, jt * 1024 : (jt + 1) * 1024], zt[:NK, :], bfk)
        fkc = fkeys[:, jt * 1024 : (jt + 1) * 1024].rearrange(
            "k (pr u c) -> k u pr c", u=2, c=128
        )
        dq.dma_start(
            out=fkeys2[0:NK, jt * 512 : (jt + 1) * 512].rearrange(
                "k (pr c) -> k pr c", c=128
            ),
            in_=fkc[:, 0],
        )
        dq.dma_start(
            out=fkeys2[NK:, jt * 512 : (jt + 1) * 512].rearrange(
                "k (pr c) -> k pr c", c=128
            ),
            in_=fkc[:, 1],
        )

    def emit_hk_chunk(jt, dq):  # 1024 cols of hkeys2 + bottom-half duplication
        zt = ztile()
        for h in range(2):
            j = 2 * jt + h
            nc.tensor.matmul(
                zt[:NK, h * 512 : (h + 1) * 512],
                wqkT,
                queryT[j // 2][:, (j % 2) * 512 : (j % 2) * 512 + 512],
                start=True,
                stop=True,
            )
        bias_copy(hkeys2[0:NK, jt * 1024 : (jt + 1) * 1024], zt[:NK, :], bqk)
        dq.dma_start(
            out=hkeys2[NK:, jt * 1024 : (jt + 1) * 1024],
            in_=hkeys2[0:NK, jt * 1024 : (jt + 1) * 1024],
        )

    emit_fk_chunk(0, dq2)  # qb0-critical dups on the ACT queue (free
    emit_hk_chunk(0, dq2)  # after ~4us); rest behind field2/3 on SP
    emit_fk_chunk(1, nc.sync)
    emit_hk_chunk(1, nc.sync)
    emit_fk_chunk(2, nc.sync)
    emit_fk_chunk(3, nc.sync)

    def emit_fvt_group(g):
        # value-projections for l-tiles 8g..8g+7; group g reads field chunk g
        zt = ztile()
        for j in range(8):
            nc.tensor.matmul(
                zt[:, j * 64 : (j + 1) * 64],
                fieldT[g][:, j * 128 : (j + 1) * 128],
                wfvT,
                start=True,
                stop=True,
            )
        if g % 2 == 0:
            nc.scalar.activation(
                out=fvT[:, g * 8 : (g + 1) * 8, 1 : NV + 1],
                in_=zt[:, 0:512].rearrange("p (a b) -> p a b", b=NV),
                func=AF.Identity,
            )
        else:
            nc.vector.tensor_copy(
                out=fvT[:, g * 8 : (g + 1) * 8, 1 : NV + 1],
                in_=zt[:, 0:512].rearrange("p (a b) -> p a b", b=NV),
            )

    # ---- main attention loop -------------------------------------------
    wconst = None
    if ABL == "zaconly":
        wconst = p["const"].tile([128, 2 * QB], bf16, tag="wconst")
        nc.vector.memset(wconst, 0.5)
    epi = [None]  # deferred epilogue state: (acc, q0)

    def emit_epilogue(last=False):
        if epi[0] is None:
            return
        acc, q0 = epi[0]
        epi[0] = None
        # all-65-row epilogue: row 0 carries den*r (ignored); engine partition
        # windows must start at 0/32/64/96, but the final DMA can read rows
        # 1..64 freely
        r = p["ep"].tile([1, QB], f32, tag="r")
        nc.vector.reciprocal_approx_fast(out=r, in_=acc[0:1, :])
        zb = ztile()
        nc.tensor.matmul(zb[: NV + 1, 0:QB], ones65, r, start=True, stop=True)
        bcs = p["ep"].tile([NV + 1, QB], f32, tag="bcs")
        nc.scalar.activation(out=bcs, in_=zb[: NV + 1, 0:QB], func=AF.Identity)
        y1 = p["ep"].tile([NV + 1, QB], f32, tag="y1")
        nc.vector.tensor_mul(y1, acc, bcs)
        y2 = p["ep"].tile([NV + 1, QB], f32, tag="y2")
        if last or not GPS:
            # tail-latency-critical: keep the bias-add on the fast ACT
            nc.scalar.activation(out=y2, in_=y1, func=AF.Identity, bias=bfv65)
        else:
            # idle Pool/GPSIMD engine takes the SBUF->SBUF bias-add
            nc.gpsimd.tensor_scalar(
                out=y2, in0=y1, scalar1=bfv65, scalar2=None, op0=ALU.add
            )
        nc.sync.dma_start(out=io["y"][:, q0 : q0 + QB], in_=y2[1 : NV + 1, :])

    for qb in range(NQB):
        q0 = qb * QB
        acc = p["acc"].tile([NVC, QB], f32, tag="acc", bufs=1 if FWL else None)

        def emit_acc(pr, w):
            nc.tensor.matmul(
                acc, fvT[:, 2 * pr, :], w[:, 0:QB], start=(pr == 0), stop=False
            )
            nc.tensor.matmul(
                acc,
                fvT[:, 2 * pr + 1, :],
                w[:, QB : 2 * QB],
                start=False,
                stop=(pr == NPAIR - 1),
            )

        pend = []
        for pr in range(NPAIR):
            zt = ztile()
            if PACK:
                nc.tensor.matmul(
                    zt[:, 0:QB],
                    fkeys2[0:NK, pr * 128 : (pr + 1) * 128],
                    hkeys2[0:NK, q0 : q0 + QB],
                    start=True,
                    stop=True,
                )
                nc.tensor.matmul(
                    zt[:, QB : 2 * QB],
                    fkeys2[NK:, pr * 128 : (pr + 1) * 128],
                    hkeys2[NK:, q0 : q0 + QB],
                    start=True,
                    stop=True,
                    tile_position=(64, 0),
                )
            else:
                nc.tensor.matmul(
                    zt[:, 0:QB],
                    fkeys[:, (2 * pr) * 128 : (2 * pr + 1) * 128],
                    hkeys2[0:NK, q0 : q0 + QB],
                    start=True,
                    stop=True,
                )
                nc.tensor.matmul(
                    zt[:, QB : 2 * QB],
                    fkeys[:, (2 * pr + 1) * 128 : (2 * pr + 2) * 128],
                    hkeys2[0:NK, q0 : q0 + QB],
                    start=True,
                    stop=True,
                )
            if qb == 0 and pr % 4 == 0:
                # interleave the value-projections into the first q-block;
                # group pr//4 is consumed by acc(pr) DEPTH iterations later
                emit_fvt_group(pr // 4)
            if ABL == "zaconly":
                w = wconst
            elif pr in ACT_SET:
                w = p["wA"].tile([128, 2 * QB], bf16, tag="wA")
                nc.scalar.activation(out=w, in_=zt, func=AF.Exp, scale=float(SCALE))
            else:
                wi = p["wB"].tile([128, 2 * QB], dt.int16, tag="wB")
                nc.vector.tensor_scalar(
                    out=wi,
                    in0=zt,
                    scalar1=A_CONST,
                    scalar2=B_CONST,
                    op0=ALU.mult,
                    op1=ALU.add,
                )
                w = wi.bitcast(bf16)
            pend.append((pr, w))
            if pr == 0 and EPID:
                # previous q-block's epilogue: emitted here so its ACT/DVE ops
                # sit between exp ops in the FIFOs instead of stalling them
                emit_epilogue()
            if ABL == "noacc" and pr not in (0, NPAIR - 1):
                pend.pop()  # drop current pair: only pairs 0/15 accumulate
            if len(pend) > DEPTH:
                emit_acc(*pend.pop(0))
        for e in pend:
            emit_acc(*e)
        epi[0] = (acc, q0)
        if DUMP and qb == 0:
            dacc = p["ep"].tile([65, QB], f32, tag="dacc")
            nc.vector.tensor_copy(out=dacc, in_=acc)
            nc.sync.dma_start(out=io["d_acc"], in_=dacc)
        if not EPID:
            emit_epilogue(last=(qb == NQB - 1))
    emit_epilogue(last=True)
    if DUMP:
        dfk = p["big"].tile([128, NPAIR * 128], f32, tag="dfk")
        nc.vector.tensor_copy(out=dfk, in_=fkeys2)
        nc.sync.dma_start(out=io["d_fk2"], in_=dfk)
        dhk = p["big"].tile([128, LQS], f32, tag="dhk")
        nc.vector.tensor_copy(out=dhk, in_=hkeys2)
        nc.sync.dma_start(out=io["d_hk2"], in_=dhk)
        dfv = p["big"].tile([128, NLT * (NV + 1)], f32, tag="dfv")
        nc.vector.tensor_copy(out=dfv, in_=fvT.rearrange("p a b -> p (a b)"))
        nc.sync.dma_start(out=io["d_fvT"], in_=dfv)


def build_nc(reps=1):
    nc = bacc.Bacc("TRN2", target_bir_lowering=False, debug=False)
    io = {
        "field": nc.dram_tensor("field", [NF, LF], dt.float32r, kind="ExternalInput").ap(),
        "query": nc.dram_tensor("query", [NF, LQS], dt.float32r, kind="ExternalInput").ap(),
        "wpack": nc.dram_tensor("wpack", [NF, 3 * NK], dt.float32r, kind="ExternalInput").ap(),
        "bpack": nc.dram_tensor("bpack", [NV + 1, 3], dt.float32, kind="ExternalInput").ap(),
        "y": nc.dram_tensor("y", [NV, LQS], dt.float32, kind="ExternalOutput").ap(),
    }
    if _os.environ.get("KDUMP", "0") == "1":
        io["d_fk2"] = nc.dram_tensor("d_fk2", [128, NPAIR * 128], dt.float32, kind="ExternalOutput").ap()
        io["d_hk2"] = nc.dram_tensor("d_hk2", [128, LQS], dt.float32, kind="ExternalOutput").ap()
        io["d_fvT"] = nc.dram_tensor("d_fvT", [128, NLT * (NV + 1)], dt.float32, kind="ExternalOutput").ap()
        io["d_acc"] = nc.dram_tensor("d_acc", [65, QB], dt.float32, kind="ExternalOutput").ap()
    with tile.TileContext(nc) as tc:
        with ExitStack() as ctx:
            p = {
                "const": ctx.enter_context(tc.tile_pool(name="const", bufs=1)),
                "big": ctx.enter_context(tc.tile_pool(name="big", bufs=2)),
                "wA": ctx.enter_context(tc.tile_pool(name="wA", bufs=4)),
                "wB": ctx.enter_context(tc.tile_pool(name="wB", bufs=4)),
                "ep": ctx.enter_context(tc.tile_pool(name="ep", bufs=2)),
                "z": ctx.enter_context(tc.tile_pool(name="z", bufs=3, space="PSUM")),
                "acc": ctx.enter_context(tc.tile_pool(name="acc", bufs=2, space="PSUM")),
            }
            for _ in range(reps):
                emit_body(nc, tc, io, p)
    nc.compile()
    return nc


def make_in_maps(field, query, W_fk, b_fk, W_fv, b_fv, W_qk, b_qk):
    field = np.asarray(field, dtype=np.float32)
    query = np.asarray(query, dtype=np.float32)
    com = {
        "wpack": np.ascontiguousarray(
            np.concatenate(
                [
                    np.asarray(W_fk, np.float32).T,
                    np.asarray(W_qk, np.float32).T,
                    np.asarray(W_fv, np.float32).T,
                ],
                axis=1,
            )
        ),
        "bpack": np.ascontiguousarray(
            np.stack(
                [
                    np.concatenate([np.asarray(b_fk, np.float32), [0.0]]),
                    np.concatenate([np.asarray(b_qk, np.float32), [0.0]]),
                    np.concatenate([[0.0], np.asarray(b_fv, np.float32)]),
                ],
                axis=1,
            ).astype(np.float32)
        ),
    }
    in_maps = []
    for c in range(NCORES):
        b, h = divmod(c, QSH)
        in_maps.append(
            {
                "field": np.ascontiguousarray(field[b]),
                "query": np.ascontiguousarray(query[b, :, h * LQS : (h + 1) * LQS]),
                **com,
            }
        )
    return in_maps


def gather(results):
    y = np.empty((B, NV, LQ), np.float32)
    for c in range(NCORES):
        b, h = divmod(c, QSH)
        y[b, :, h * LQS : (h + 1) * LQS] = results[c]["y"]
    return y


_NC_CACHE = {}


def get_nc(reps=1):
    if reps not in _NC_CACHE:
        _NC_CACHE[reps] = build_nc(reps)
    return _NC_CACHE[reps]


def kernel(field, query, W_fk, b_fk, W_fv, b_fv, W_qk, b_qk):
    nc = get_nc(1)
    in_maps = make_in_maps(field, query, W_fk, b_fk, W_fv, b_fv, W_qk, b_qk)
    res = run_bass_kernel_spmd(nc, in_maps, core_ids=list(range(NCORES)))
    return gather(res.results)


# revision 21
# speedup vs baseline: 2.1339x; 2.1339x over previous
"""Trainium2 Bass kernel for nn_Attention_53386443489626.

Math (per batch b):
    fkeys = W_fk @ field + b_fk          [NK, Lf]
    fvals = W_fv @ field + b_fv          [NV, Lf]
    hkeys = W_qk @ query + b_qk          [NK, Lq]
    z     = fkeys^T @ hkeys / sqrt(NK)   [Lf, Lq]
    w     = exp(clip(z, -30, 30))        (clip is a no-op: max |z| ~ 9.4)
    w     = w / sum_l w
    y     = fvals @ w                    [NV, Lq]

Single-pass accumulation (clip bounds the exponent, so no running max):
    acc[v,q] = sum_l fvals0[v,l] * wu[l,q],  den[q] = sum_l wu[l,q]
    y[v,q]   = acc[v,q] / den[q] + b_fv[v]
den comes from the same matmul stream via a ones column PREPENDED to the
transposed values (fvT[:, lt, 0] = 1) so it lands on acc partition 0, where
reciprocal_approx_fast can read it (the custom DVE op ignores a nonzero
partition-base offset on HW).

Sharding: 8 cores = 4 batches x 2 query-halves; normalization is over Lf so
no cross-core communication.

Performance structure (v5):
  * The exp of 8.4M score elements/core is split across TWO engines per
    l-tile pair: ACT pairs use native activation(Exp); DVE pairs use a
    one-op Schraudolph bit-trick exp,
        i16 = int16(z * 2^7*log2(e)/8 + 2^7*(127-c)),
    whose bits read as bf16 approximate exp(z/8) to ~3.5% max rel err
    (end-to-end ~1e-2 vs the 2e-2 tolerance; HW-validated).
  * ALL steady-loop matmuls run in the SAME 64x128 row-tiled PE mode --
    switching tiling modes drains the PE array (~9us/q-block measured when
    the K=64 score matmuls alternated with full-128 acc matmuls):
      - score z: two concurrent K=64 row-group matmuls (tiles T0/T8)
      - acc: each l-tile's K=128 contraction is SPLIT into two concurrent
        K=64 halves (T0 -> accA, T8 -> accB; separate PSUM banks), merged
        in the epilogue (s = accA + accB)
      - the 1/den broadcast matmul is zero-padded to K=64
  * Keys/values/weights in bf16 (validated: adds <1e-3 on top of the
    approx-exp error; halves duplication-DMA bytes; FWL weight loads).
  * Each DMA costs a ~650ns serialized HWDGE slot: inputs are chunked and
    spread over both queues (SP + ACT) in need-order; consts are packed
    into two DMAs.
  * Epilogue is emitted at the top of the NEXT q-block (engine FIFOs stay
    busy); its bias-add runs on the otherwise-idle GPSIMD engine.
"""

import numpy as np
from contextlib import ExitStack

try:
    import concourse  # noqa: F401
except ImportError:  # pragma: no cover
    import sys

    sys.path.insert(0, "/opt/trn_rl_repo")

import concourse.bacc as bacc
import concourse.mybir as mybir
import concourse.tile as tile
from concourse.bass_utils import run_bass_kernel_spmd

dt = mybir.dt
AF = mybir.ActivationFunctionType
ALU = mybir.AluOpType

B, NF, NK, NV = 4, 128, 64, 64
LF, LQ = 4096, 4096
import os as _os

NCORES = 8
QSH = NCORES // B  # query shards per batch = 2
LQS = LQ // QSH  # per-core query length = 2048
NLT = LF // 128  # 32 l-tiles
NPAIR = NLT // 2  # 16 l-tile pairs
QB = 512  # query columns per accumulation block
NQB = LQS // QB  # 4
SCALE = 1.0 / np.sqrt(NK)  # 0.125

N_ACT = int(_os.environ.get("KACT", "9"))  # pairs on ACT (rest on DVE)
DEPTH = int(_os.environ.get("KDEPTH", "2"))  # acc-matmul pipeline depth
Q2 = _os.environ.get("KQ2", "1") == "1"  # use the ACT hwdge queue too
GPS = _os.environ.get("KGPS", "1") == "1"  # y2 bias-add on gpsimd
ABL = _os.environ.get("KABL", "")  # timing ablation: zaconly
SCHRAUD_C = float(_os.environ.get("KC", "0.0434"))
A_CONST = float(2.0**7 * np.log2(np.e) * SCALE)
B_CONST = float(2.0**7 * (127.0 - SCHRAUD_C) + 0.5)

ACT_SET = {(i * NPAIR) // N_ACT for i in range(N_ACT)} if N_ACT > 0 else set()


def emit_body(nc, tc, io, p):
    f32 = dt.float32
    f32r = dt.float32r
    bf16 = dt.bfloat16

    # ---- constants ------------------------------------------------------
    wpack = p["const"].tile([NF, 3 * NK], f32r, tag="wpack")
    bpack = p["const"].tile([NV + 1, 3], f32, tag="bpack")
    wfkT = wpack[:, 0:NK]
    wqkT = wpack[:, NK : 2 * NK]
    wfvT = wpack[:, 2 * NK : 3 * NK]
    bfk = bpack[0:NK, 0:1]
    bqk = bpack[0:NK, 1:2]
    bfv65 = bpack[:, 2:3]  # [65,1]: row 0 pad, rows 1..64 = b_fv
    # onesP[0,:] = 1, rows 1..63 = 0: K=64 stationary for the 1/den
    # broadcast so it shares the 64x128 PE tiling mode (no mode-switch drain)
    onesP = p["const"].tile([NK, NV + 1], f32, tag="onesP")
    nc.vector.memset(onesP, 0.0)
    nc.vector.memset(onesP[0:1, :], 1.0)
    # single moving tile for the broadcast matmul: row 0 = 1/den (written per
    # epilogue), rows 1..63 stay zero from this one-time memset so the
    # zero-padded stationary rows never touch uninitialized (possibly NaN) SBUF
    r64 = p["const"].tile([NK, QB], f32, tag="r64")
    nc.vector.memset(r64, 0.0)

    # ---- inputs (chunked; both DMA queues; need-order) ------------------
    fieldT = [
        p["big"].tile([NF, 1024], f32r, tag=f"field{c}", name=f"field{c}")
        for c in range(LF // 1024)
    ]
    queryT = [
        p["big"].tile([NF, 1024], f32r, tag=f"query{c}", name=f"query{c}")
        for c in range(LQS // 1024)
    ]

    def half_loads(dq, tiles, src, idx):
        for c, h in idx:
            dq.dma_start(
                out=tiles[c][:, h * 512 : (h + 1) * 512],
                in_=src[:, c * 1024 + h * 512 : c * 1024 + (h + 1) * 512],
            )

    dq2 = nc.scalar if Q2 else nc.sync
    half_loads(nc.sync, fieldT, io["field"], [(0, 0)])
    half_loads(dq2, queryT, io["query"], [(0, 0)])
    nc.sync.dma_start(out=wpack, in_=io["wpack"])
    dq2.dma_start(out=bpack, in_=io["bpack"])
    half_loads(nc.sync, fieldT, io["field"], [(0, 1)])
    half_loads(dq2, queryT, io["query"], [(0, 1)])
    half_loads(nc.sync, fieldT, io["field"], [(1, 0), (1, 1)])
    half_loads(dq2, queryT, io["query"], [(1, 0), (1, 1)])
    half_loads(nc.sync, fieldT, io["field"], [(2, 0), (2, 1), (3, 0), (3, 1)])

    # fkeys2[0:64, pr*128+i]   = fkeys[k, (2*pr)*128+i]   (even l-tiles, top)
    # fkeys2[64:128, pr*128+i] = fkeys[k, (2*pr+1)*128+i] (odd l-tiles, bottom)
    fkeys2 = p["big"].tile([128, NPAIR * 128], bf16, tag="fkeys")
    hkeys2 = p["big"].tile([128, LQS], bf16, tag="hkeys")  # duplicated halves
    fkeys = p["big"].tile([NK, LF], bf16, tag="fkeysflat")
    fvT = p["big"].tile([128, NLT, NV + 1], bf16, tag="fvT")
    nc.vector.memset(fvT[:, :, 0:1], 1.0)  # denominator column (col 0)

    # trigger the ACT exp table load at t~0 (hide its ~2.7us under the DMAs)
    dume = p["const"].tile([1, 1], f32, tag="dume")
    nc.scalar.activation(out=dume, in_=onesP[0:1, 0:1], func=AF.Exp)

    def ztile():
        return p["z"].tile([128, 1024], f32, tag="z", name="zt")

    # ---- projections ----------------------------------------------------
    copy_eng = [0]

    def bias_copy(out, in_, bias):
        if copy_eng[0] % 2 == 0:
            nc.scalar.activation(out=out, in_=in_, func=AF.Identity, bias=bias)
        else:
            nc.vector.tensor_scalar(
                out=out, in0=in_, scalar1=bias, scalar2=None, op0=ALU.add
            )
        copy_eng[0] += 1

    def emit_fk_chunk(jt, dq):  # 1024 cols of fkeys + its fkeys2 duplication
        zt = ztile()
        for h in range(2):
            j = 2 * jt + h
            nc.tensor.matmul(
                zt[:NK, h * 512 : (h + 1) * 512],
                wfkT,
                fieldT[j // 2][:, (j % 2) * 512 : (j % 2) * 512 + 512],
                start=True,
                stop=True,
            )
        bias_copy(fkeys[:, jt * 1024 : (jt + 1) * 1024], zt[:NK, :], bfk)
        fkc = fkeys[:, jt * 1024 : (jt + 1) * 1024].rearrange(
            "k (pr u c) -> k u pr c", u=2, c=128
        )
        dq.dma_start(
            out=fkeys2[0:NK, jt * 512 : (jt + 1) * 512].rearrange(
                "k (pr c) -> k pr c", c=128
            ),
            in_=fkc[:, 0],
        )
        dq.dma_start(
            out=fkeys2[NK:, jt * 512 : (jt + 1) * 512].rearrange(
                "k (pr c) -> k pr c", c=128
            ),
            in_=fkc[:, 1],
        )

    def emit_hk_chunk(jt, dq):  # 1024 cols of hkeys2 + bottom-half duplication
        zt = ztile()
        for h in range(2):
            j = 2 * jt + h
            nc.tensor.matmul(
                zt[:NK, h * 512 : (h + 1) * 512],
                wqkT,
                queryT[j // 2][:, (j % 2) * 512 : (j % 2) * 512 + 512],
                start=True,
                stop=True,
            )
        bias_copy(hkeys2[0:NK, jt * 1024 : (jt + 1) * 1024], zt[:NK, :], bqk)
        dq.dma_start(
            out=hkeys2[NK:, jt * 1024 : (jt + 1) * 1024],
            in_=hkeys2[0:NK, jt * 1024 : (jt + 1) * 1024],
        )

    def emit_fvt_group(g):
        # value-projections for l-tiles 8g..8g+7; group g reads field chunk g
        zt = ztile()
        for j in range(8):
            nc.tensor.matmul(
                zt[:, j * 64 : (j + 1) * 64],
                fieldT[g][:, j * 128 : (j + 1) * 128],
                wfvT,
                start=True,
                stop=True,
            )
        if g % 2 == 0:
            nc.scalar.activation(
                out=fvT[:, g * 8 : (g + 1) * 8, 1 : NV + 1],
                in_=zt[:, 0:512].rearrange("p (a b) -> p a b", b=NV),
                func=AF.Identity,
            )
        else:
            nc.vector.tensor_copy(
                out=fvT[:, g * 8 : (g + 1) * 8, 1 : NV + 1],
                in_=zt[:, 0:512].rearrange("p (a b) -> p a b", b=NV),
            )

    emit_fk_chunk(0, dq2)  # qb0-critical dups on the ACT queue
    emit_hk_chunk(0, dq2)
    emit_fk_chunk(1, nc.sync)
    emit_hk_chunk(1, nc.sync)
    emit_fk_chunk(2, nc.sync)
    emit_fk_chunk(3, nc.sync)
    # fvt stays in the prologue: its K=128 matmuls would force PE tiling-mode
    # switches (array drains) if interleaved with the 64x128-mode main loop
    for g in range(4):
        emit_fvt_group(g)

    wconst = None
    if ABL == "zaconly":
        wconst = p["const"].tile([128, 2 * QB], bf16, tag="wconst")
        nc.vector.memset(wconst, 0.5)

    # ---- main attention loop (all matmuls in 64x128 row-tiled mode) -----
    epi = [None]  # deferred epilogue state: (s_tile, q0) between pre and post

    def emit_epilogue_pre(accA, accB, q0):
        # reciprocal chain: runs while the next q-block's first z pairs flow
        aS = p["ep"].tile([NV + 1, QB], f32, tag="aS")
        nc.scalar.activation(out=aS, in_=accA, func=AF.Identity)
        s = p["ep"].tile([NV + 1, QB], f32, tag="s")
        nc.vector.tensor_add(s, aS, accB)
        nc.vector.reciprocal_approx_fast(out=r64[0:1, :], in_=s[0:1, :])
        epi[0] = (s, q0)

    def emit_epilogue_post(last=False):
        if epi[0] is None:
            return
        s, q0 = epi[0]
        epi[0] = None
        zb = ztile()
        nc.tensor.matmul(zb[: NV + 1, 0:QB], onesP, r64, start=True, stop=True)
        bcs = p["ep"].tile([NV + 1, QB], f32, tag="bcs")
        nc.scalar.activation(out=bcs, in_=zb[: NV + 1, 0:QB], func=AF.Identity)
        y1 = p["ep"].tile([NV + 1, QB], f32, tag="y1")
        nc.vector.tensor_mul(y1, s, bcs)
        y2 = p["ep"].tile([NV + 1, QB], f32, tag="y2")
        if last or not GPS:
            nc.scalar.activation(out=y2, in_=y1, func=AF.Identity, bias=bfv65)
        else:
            nc.gpsimd.tensor_scalar(
                out=y2, in0=y1, scalar1=bfv65, scalar2=None, op0=ALU.add
            )
        nc.sync.dma_start(out=io["y"][:, q0 : q0 + QB], in_=y2[1 : NV + 1, :])

    prev = [None]  # (accA, accB, q0) of the previous q-block
    for qb in range(NQB):
        q0 = qb * QB
        if prev[0] is not None:
            emit_epilogue_pre(*prev[0])  # before the acc rings recycle
        accA = p["accA"].tile([NV + 1, QB], f32, tag="accA")
        accB = p["accB"].tile([NV + 1, QB], f32, tag="accB")

        def emit_acc(pr, w):
            for j in range(2):
                lt = 2 * pr + j
                wsl = w[:, j * QB : (j + 1) * QB]
                nc.tensor.matmul(
                    accA,
                    fvT[0:NK, lt, :],
                    wsl[0:NK, :],
                    start=(lt == 0),
                    stop=(lt == NLT - 1),
                )
                nc.tensor.matmul(
                    accB,
                    fvT[NK:, lt, :],
                    wsl[NK:, :],
                    start=(lt == 0),
                    stop=(lt == NLT - 1),
                    tile_position=(64, 0),
                )

        pend = []
        for pr in range(NPAIR):
            zt = ztile()
            nc.tensor.matmul(
                zt[:, 0:QB],
                fkeys2[0:NK, pr * 128 : (pr + 1) * 128],
                hkeys2[0:NK, q0 : q0 + QB],
                start=True,
                stop=True,
            )
            nc.tensor.matmul(
                zt[:, QB : 2 * QB],
                fkeys2[NK:, pr * 128 : (pr + 1) * 128],
                hkeys2[NK:, q0 : q0 + QB],
                start=True,
                stop=True,
                tile_position=(64, 0),
            )
            if ABL == "zaconly":
                w = wconst
            elif pr in ACT_SET:
                w = p["wA"].tile([128, 2 * QB], bf16, tag="wA")
                nc.scalar.activation(out=w, in_=zt, func=AF.Exp, scale=float(SCALE))
            else:
                wi = p["wB"].tile([128, 2 * QB], dt.int16, tag="wB")
                nc.vector.tensor_scalar(
                    out=wi,
                    in0=zt,
                    scalar1=A_CONST,
                    scalar2=B_CONST,
                    op0=ALU.mult,
                    op1=ALU.add,
                )
                w = wi.bitcast(bf16)
            pend.append((pr, w))
            if pr == 1:
                # the broadcast matmul lands here in the PE stream: the
                # reciprocal chain has had two z-pairs' time to finish
                emit_epilogue_post(last=(qb == NQB - 1))
            if len(pend) > DEPTH:
                emit_acc(*pend.pop(0))
        for e in pend:
            emit_acc(*e)
        prev[0] = (accA, accB, q0)
    emit_epilogue_pre(*prev[0])
    emit_epilogue_post(last=True)


def build_nc(reps=1):
    nc = bacc.Bacc("TRN2", target_bir_lowering=False, debug=False)
    io = {
        "field": nc.dram_tensor("field", [NF, LF], dt.float32r, kind="ExternalInput").ap(),
        "query": nc.dram_tensor("query", [NF, LQS], dt.float32r, kind="ExternalInput").ap(),
        "wpack": nc.dram_tensor("wpack", [NF, 3 * NK], dt.float32r, kind="ExternalInput").ap(),
        "bpack": nc.dram_tensor("bpack", [NV + 1, 3], dt.float32, kind="ExternalInput").ap(),
        "y": nc.dram_tensor("y", [NV, LQS], dt.float32, kind="ExternalOutput").ap(),
    }
    with tile.TileContext(nc) as tc:
        with ExitStack() as ctx:
            p = {
                "const": ctx.enter_context(tc.tile_pool(name="const", bufs=1)),
                "big": ctx.enter_context(tc.tile_pool(name="big", bufs=2)),
                "wA": ctx.enter_context(tc.tile_pool(name="wA", bufs=4)),
                "wB": ctx.enter_context(tc.tile_pool(name="wB", bufs=4)),
                "ep": ctx.enter_context(tc.tile_pool(name="ep", bufs=2)),
                "z": ctx.enter_context(tc.tile_pool(name="z", bufs=3, space="PSUM")),
                "accA": ctx.enter_context(tc.tile_pool(name="accA", bufs=1, space="PSUM")),
                "accB": ctx.enter_context(tc.tile_pool(name="accB", bufs=1, space="PSUM")),
            }
            for _ in range(reps):
                emit_body(nc, tc, io, p)
    nc.compile()
    return nc


def make_in_maps(field, query, W_fk, b_fk, W_fv, b_fv, W_qk, b_qk):
    field = np.asarray(field, dtype=np.float32)
    query = np.asarray(query, dtype=np.float32)
    com = {
        "wpack": np.ascontiguousarray(
            np.concatenate(
                [
                    np.asarray(W_fk, np.float32).T,
                    np.asarray(W_qk, np.float32).T,
                    np.asarray(W_fv, np.float32).T,
                ],
                axis=1,
            )
        ),
        "bpack": np.ascontiguousarray(
            np.stack(
                [
                    np.concatenate([np.asarray(b_fk, np.float32), [0.0]]),
                    np.concatenate([np.asarray(b_qk, np.float32), [0.0]]),
                    np.concatenate([[0.0], np.asarray(b_fv, np.float32)]),
                ],
                axis=1,
            ).astype(np.float32)
        ),
    }
    in_maps = []
    for c in range(NCORES):
        b, h = divmod(c, QSH)
        in_maps.append(
            {
                "field": np.ascontiguousarray(field[b]),
                "query": np.ascontiguousarray(query[b, :, h * LQS : (h + 1) * LQS]),
                **com,
            }
        )
    return in_maps


def gather(results):
    y = np.empty((B, NV, LQ), np.float32)
    for c in range(NCORES):
        b, h = divmod(c, QSH)
        y[b, :, h * LQS : (h + 1) * LQS] = results[c]["y"]
    return y


_NC_CACHE = {}


def get_nc(reps=1):
    if reps not in _NC_CACHE:
        _NC_CACHE[reps] = build_nc(reps)
    return _NC_CACHE[reps]


def kernel(field, query, W_fk, b_fk, W_fv, b_fv, W_qk, b_qk):
    nc = get_nc(1)
    in_maps = make_in_maps(field, query, W_fk, b_fk, W_fv, b_fv, W_qk, b_qk)
    res = run_bass_kernel_spmd(nc, in_maps, core_ids=list(range(NCORES)))
    return gather(res.results)
